# revision 8
# baseline (speedup 1.0000x reference)
"""AttentionRGCN (3x RGCN + GAT) Trainium2 Bass kernel, 8-core SPMD.

Strategy: shard nodes (dst) across 8 cores; edges live with their dst core.
Per dst-tile (128 nodes) aggregation via one-hot matmuls:
  aggT[f, d] (per relation) = sum_e x[src_e, f] * S[e, d],
  S[e, d] = (d == dst_local[e]) * inv_deg[e]
One fused DVE tensor_scalar builds each S slice:
  S = (iota is_equal dstl_col) mult inv_col     (per-partition scalars)
Edges are packed per-core CONTIGUOUSLY per (tile, half) run (sorted by rel),
padded to the cross-core max rounded to 128; relation boundaries drift per
core, so each (chunk, rel) overlap gets its own sentinel-masked S slice.
Per-edge source rows fetched with gpsimd.dma_gather (fp16 256B rows, int16
idx, half-split tables for the 32k index range), one gather per run.
Between layers: AllGather. GAT: self-loops are NOT gathered (self term is
computed directly from local features); attention logits from gathered
[x | alpha_src] ext rows + local alpha_dst via transposed one-hot matmul;
segment softmax without max-subtraction; denominator applied per-dst AFTER
aggregation via a diagonal matmul.

Data path is fp16; accumulation (PSUM), LayerNorm statistics and epilogue
math stay fp32. PSUM->SBUF copies ride the idle Scalar (ACT) engine.
"""
import sys
sys.path.insert(0, "/opt/trn_rl_repo")
import numpy as np

import concourse.bass as bass
import concourse.bacc as bacc
import concourse.mybir as mybir
import concourse.tile as tile
from concourse.bass_utils import run_bass_kernel_spmd


def bc(ap_obj, dims):
    """Custom broadcast AP: keep partition dim of ap_obj, replace free dims."""
    return bass.AP(ap_obj.tensor, ap_obj.offset, [list(ap_obj.ap[0])] + dims)

F32 = mybir.dt.float32
F16 = mybir.dt.float16
I16 = mybir.dt.int16
AF = mybir.ActivationFunctionType
OP = mybir.AluOpType

NEG = 0.1
LN_EPS = 1e-5
GAT_NEG = 0.2
SENT = 999.0


def default_cfg():
    return dict(N=50000, NP=50176, E=600000, R=8, B=8, D=128, H=4,
                CORES=8, PER=6272, TILES=49, HALF=25088, EXTD=256)


# ----------------------------------------------------------------------------
# Host-side graph preprocessing
# ----------------------------------------------------------------------------

def wrap_idx(flat: np.ndarray) -> np.ndarray:
    """int16 flat idx list (len mult of 128) -> [128, len/16] wrapped layout."""
    n = len(flat)
    assert n % 128 == 0
    w = flat.astype(np.int16).reshape(n // 16, 16).T  # [16, n/16]
    return np.tile(w, (8, 1))


def build_graph_plan(cfg, edge_index, edge_type):
    """Per-core contiguous packing with shared run widths and union slices.

    plan:
      rplan: per tile -> list of per-half dicts:
          (h, chunk_base, nch, slices=[(rel, j, col, start, stop), ...])
      gplan: per tile -> list of (h, chunk_base, nch)
      r_total_ch, g_total_ch, r_total_sl, rwin, gwin
    per_core[c]: ridx, rdstl, rinv (per-slice cols), gidx, gdstl
    """
    N, NP, R = cfg["N"], cfg["NP"], cfg["R"]
    CORES, PER, TILES, HALF = cfg["CORES"], cfg["PER"], cfg["TILES"], cfg["HALF"]
    src, dst = edge_index[0].astype(np.int64), edge_index[1].astype(np.int64)
    rel = edge_type.astype(np.int64)

    deg = np.bincount(rel * N + dst, minlength=R * N).astype(np.float32)
    inv_tab = np.float32(1.0) / np.maximum(deg, np.float32(1.0))

    core_of = dst // PER
    NRUN = TILES * 2

    # ---- per-core sorted edge arrays ----
    r_edges = []  # per core: (runkey, rel, src, dstl128, inv) sorted
    g_edges = []  # per core: (runkey, src, dstl128) sorted (no self loops)
    for c in range(CORES):
        m = core_of == c
        s_c, d_c, r_c = src[m], dst[m], rel[m]
        dl = d_c - c * PER
        t_c = dl // 128
        h_c = (s_c >= HALF).astype(np.int64)
        runk = t_c * 2 + h_c
        order = np.lexsort((r_c, runk))
        r_edges.append((runk[order], r_c[order], s_c[order],
                        (d_c[order] % 128).astype(np.float32),
                        inv_tab[r_c[order] * N + d_c[order]]))
        gorder = np.argsort(runk, kind="stable")
        g_edges.append((runk[gorder], s_c[gorder],
                        (d_c[gorder] % 128).astype(np.float32)))

    # ---- run widths ----
    rcnt = np.zeros((CORES, NRUN), np.int64)   # edges per (core, run)
    rcnt_cr = np.zeros((CORES, NRUN, R), np.int64)
    gcnt = np.zeros((CORES, NRUN), np.int64)
    for c in range(CORES):
        runk, rl = r_edges[c][0], r_edges[c][1]
        np.add.at(rcnt[c], runk, 1)
        np.add.at(rcnt_cr[c], (runk, rl), 1)
        np.add.at(gcnt[c], g_edges[c][0], 1)
    rW = (np.ceil(rcnt.max(axis=0) / 128).astype(np.int64)) * 128
    gW = (np.ceil(gcnt.max(axis=0) / 128).astype(np.int64)) * 128
    # per-core start offsets of each rel within its run (packed)
    rstart = np.cumsum(rcnt_cr, axis=2) - rcnt_cr  # [C, NRUN, R]
    rend = rstart + rcnt_cr

    # ---- chunk bases + slices ----
    rplan = [[] for _ in range(TILES)]
    gplan = [[] for _ in range(TILES)]
    r_total_ch = 0
    g_total_ch = 0
    r_total_sl = 0
    rsl_info = []   # (run, rel, j, col) in emit order
    rch_base = np.zeros(NRUN, np.int64)
    gch_base = np.zeros(NRUN, np.int64)
    for run in range(NRUN):
        t, h = run // 2, run % 2
        nch = int(rW[run] // 128)
        rch_base[run] = r_total_ch
        slices = []
        if nch:
            for r in range(R):
                act = rcnt_cr[:, run, r] > 0
                if not act.any():
                    continue
                jlo = int((rstart[act, run, r] // 128).min())
                jhi = int(((rend[act, run, r] - 1) // 128).max()) + 1
                js = list(range(jlo, jhi))
                for k, j in enumerate(js):
                    col = r_total_sl
                    r_total_sl += 1
                    slices.append((r, j, col, k == 0, k == len(js) - 1))
                    rsl_info.append((run, r, j, col))
            rplan[t].append(dict(h=h, base=r_total_ch, nch=nch,
                                 slices=slices))
            r_total_ch += nch
        gn = int(gW[run] // 128)
        gch_base[run] = g_total_ch
        if gn:
            gplan[t].append((h, g_total_ch, gn))
            g_total_ch += gn
    rwin = max((p["nch"] for tp in rplan for p in tp), default=1)
    gwin = max((g[2] for tp in gplan for g in tp), default=1)

    # ---- per-core tables ----
    per_core = []
    for c in range(CORES):
        runk, rl, s_c, dl_c, inv_c = r_edges[c]
        rbounds = np.searchsorted(runk, np.arange(NRUN + 1))
        ridx = np.zeros(r_total_ch * 128, np.int64)
        # per-run packed values
        run_dstl = np.full(r_total_ch * 128, SENT, np.float32)
        run_inv = np.zeros(r_total_ch * 128, np.float32)
        run_rel = np.full(r_total_ch * 128, -1, np.int64)
        for run in range(NRUN):
            lo, hi = rbounds[run], rbounds[run + 1]
            cnt = hi - lo
            if rW[run] == 0:
                continue
            p0 = rch_base[run] * 128
            ridx[p0:p0 + cnt] = s_c[lo:hi] - (run % 2) * HALF
            run_dstl[p0:p0 + cnt] = dl_c[lo:hi]
            run_inv[p0:p0 + cnt] = inv_c[lo:hi]
            run_rel[p0:p0 + cnt] = rl[lo:hi]
        # per-slice columns
        rdstl = np.full((r_total_sl, 128), SENT, np.float32)
        rinv = np.zeros((r_total_sl, 128), np.float32)
        for (run, r, j, col) in rsl_info:
            p0 = (rch_base[run] + j) * 128
            seg = slice(p0, p0 + 128)
            msk = run_rel[seg] == r
            rdstl[col][msk] = run_dstl[seg][msk]
            rinv[col][msk] = run_inv[seg][msk]

        grunk, gs_c, gdl_c = g_edges[c]
        gbounds = np.searchsorted(grunk, np.arange(NRUN + 1))
        gidx = np.zeros(g_total_ch * 128, np.int64)
        gdstl = np.full(g_total_ch * 128, SENT, np.float32)
        for run in range(NRUN):
            lo, hi = gbounds[run], gbounds[run + 1]
            cnt = hi - lo
            if gW[run] == 0:
                continue
            p0 = gch_base[run] * 128
            gidx[p0:p0 + cnt] = gs_c[lo:hi] - (run % 2) * HALF
            gdstl[p0:p0 + cnt] = gdl_c[lo:hi]

        per_core.append(dict(
            ridx=wrap_idx(ridx),
            rdstl=rdstl.T.copy(),   # [128, NSLICE] f32 (scalar operands)
            rinv=rinv.T.copy(),
            gidx=wrap_idx(gidx),
            gdstl=gdstl.reshape(g_total_ch, 128).T.copy(),
        ))

    plan = dict(rplan=rplan, gplan=gplan, r_total_ch=r_total_ch,
                g_total_ch=g_total_ch, r_total_sl=r_total_sl,
                rwin=rwin, gwin=gwin)
    return plan, per_core


# ----------------------------------------------------------------------------
# Weight preprocessing (host)
# ----------------------------------------------------------------------------

def prep_weights(cfg, inp):
    D, H = cfg["D"], cfg["H"]
    out = {}
    for li, pre in (("0", "r0"), ("1", "r1"), ("3", "r2")):
        W = np.einsum("rb,bio->rio", inp[pre + "_comp"], inp[pre + "_basis"])
        Wstack = np.concatenate([W[r] for r in range(cfg["R"])] +
                                [inp[pre + "_root"]], axis=1)  # [D, 9*D]
        out["w" + li] = Wstack.astype(np.float16)
        out["bias" + li] = np.tile(inp[pre + "_bias"][None, :], (128, 1)).astype(np.float32)
    gw = inp["gat_w"]  # [D, H*D]
    out["gatw"] = (gw / H).astype(np.float16)
    U = np.zeros((D, 2 * H), np.float32)
    for h in range(H):
        Wh = gw[:, h * D:(h + 1) * D]
        U[:, h] = Wh @ inp["gat_asrc"][h]
        U[:, H + h] = Wh @ inp["gat_adst"][h]
    out["gatu"] = U.astype(np.float16)
    out["gbias"] = np.tile(inp["gat_bias"][None, :], (128, 1)).astype(np.float32)
    for k in ("ln0", "ln1", "ln2"):
        out[k + "g"] = np.tile(inp[k + "_g"][None, :], (128, 1)).astype(np.float32)
        out[k + "b"] = np.tile(inp[k + "_b"][None, :], (128, 1)).astype(np.float32)
    out["iota"] = np.tile(np.arange(128, dtype=np.float16)[None, :], (128, 1))
    ident = np.zeros((128, 128), np.float16)
    np.fill_diagonal(ident, 1.0)
    out["ident"] = ident
    return out


def build_in_maps(cfg, inputs, per_core):
    N, NP, CORES, PER = cfg["N"], cfg["NP"], cfg["CORES"], cfg["PER"]
    wts = prep_weights(cfg, inputs)
    x = np.asarray(inputs["x"], dtype=np.float32)
    x_pad = np.zeros((NP, cfg["D"]), np.float16)
    x_pad[:N] = x.astype(np.float16)
    in_maps = []
    for c in range(CORES):
        m = dict(wts)
        m["x_pad"] = x_pad
        m["x_own"] = x_pad[c * PER:(c + 1) * PER]
        m.update(per_core[c])
        in_maps.append(m)
    return in_maps


# ----------------------------------------------------------------------------
# Bass program
# ----------------------------------------------------------------------------

def build_nc(cfg, plan):
    N, NP, R, D, H = cfg["N"], cfg["NP"], cfg["R"], cfg["D"], cfg["H"]
    CORES, PER, TILES, HALF = cfg["CORES"], cfg["PER"], cfg["TILES"], cfg["HALF"]
    RCH, GCH = plan["r_total_ch"], plan["g_total_ch"]
    NSL = plan["r_total_sl"]
    rplan, gplan = plan["rplan"], plan["gplan"]
    EXTD = cfg["EXTD"]  # ext row fp16 elems: [x(128) | a(8) | pad]
    RWIN, GWIN = plan["rwin"], plan["gwin"]

    nc = bacc.Bacc("TRN2", target_bir_lowering=False, debug=False,
                   num_devices=CORES)

    def inp(name, shape, dt=F16):
        return nc.dram_tensor(name, shape, dt, kind="ExternalInput").ap()

    x_pad = inp("x_pad", [NP, D])
    x_own = inp("x_own", [PER, D])
    w0, w1, w3 = (inp(k, [D, (R + 1) * D]) for k in ("w0", "w1", "w3"))
    bias0, bias1, bias3 = (inp(k, [128, D], F32) for k in ("bias0", "bias1", "bias3"))
    gatw = inp("gatw", [D, H * D])
    gatu = inp("gatu", [D, 2 * H])
    gbias = inp("gbias", [128, D], F32)
    ln0g, ln0b = inp("ln0g", [128, D], F32), inp("ln0b", [128, D], F32)
    ln1g, ln1b = inp("ln1g", [128, D], F32), inp("ln1b", [128, D], F32)
    ln2g, ln2b = inp("ln2g", [128, D], F32), inp("ln2b", [128, D], F32)
    iota_in = inp("iota", [128, 128])
    ident_in = inp("ident", [128, 128])
    ridx_in = inp("ridx", [128, RCH * 8], I16)
    rdstl_in = inp("rdstl", [128, NSL], F32)
    rinv_in = inp("rinv", [128, NSL], F32)
    gidx_in = inp("gidx", [128, GCH * 8], I16)
    gdstl_in = inp("gdstl", [128, GCH], F32)

    out_dram = nc.dram_tensor("out", [PER, D], F32, kind="ExternalOutput").ap()

    xex0 = nc.dram_tensor("xex0", [NP, D], F16).ap()
    ag0_in = nc.dram_tensor("ag0_in", [PER, D], F16).ap()
    xex1 = nc.dram_tensor("xex1", [NP, D], F16, addr_space="Shared").ap()
    ag1_in = nc.dram_tensor("ag1_in", [PER, EXTD], F16).ap()
    xex2 = nc.dram_tensor("xex2", [NP, EXTD], F16, addr_space="Shared").ap()
    ag2_in = nc.dram_tensor("ag2_in", [PER, D], F16).ap()
    xex3 = nc.dram_tensor("xex3", [NP, D], F16, addr_space="Shared").ap()

    rg = [list(range(CORES))]

    with tile.TileContext(nc) as tc:
        with (
            tc.tile_pool(name="const", bufs=1) as cpool,
            tc.tile_pool(name="gath", bufs=2) as gpool,
            tc.tile_pool(name="work", bufs=2) as wpool,
            tc.tile_pool(name="slc", bufs=6) as slpool,
            tc.tile_pool(name="stage", bufs=2) as spool,
            tc.tile_pool(name="psA", bufs=2, space="PSUM") as psA,
            tc.tile_pool(name="psB", bufs=4, space="PSUM") as psB,
        ):
            # ---- load constants ----
            def ld(ap_in, shape, dt=F16, tag=None):
                t = cpool.tile(shape, dt, tag=tag)
                nc.sync.dma_start(out=t[:], in_=ap_in[:])
                return t

            iota = ld(iota_in, [128, 128], tag="c_iota")
            ident = ld(ident_in, [128, 128], tag="c_ident")
            Ws = {0: ld(w0, [D, (R + 1) * D], tag="c_w0"),
                  1: ld(w1, [D, (R + 1) * D], tag="c_w1"),
                  3: ld(w3, [D, (R + 1) * D], tag="c_w3")}
            LNg = {0: ld(ln0g, [128, D], F32, tag="c_l0g"),
                   1: ld(ln1g, [128, D], F32, tag="c_l1g"),
                   2: ld(ln2g, [128, D], F32, tag="c_l2g")}
            LNb = {0: ld(ln0b, [128, D], F32, tag="c_l0b"),
                   1: ld(ln1b, [128, D], F32, tag="c_l1b"),
                   2: ld(ln2b, [128, D], F32, tag="c_l2b")}
            BIAS = {0: ld(bias0, [128, D], F32, tag="c_b0"),
                    1: ld(bias1, [128, D], F32, tag="c_b1"),
                    3: ld(bias3, [128, D], F32, tag="c_b3")}
            gw_sb = ld(gatw, [D, H * D], tag="c_gw")
            gu_sb = ld(gatu, [D, 2 * H], tag="c_gu")
            gb_sb = ld(gbias, [128, D], F32, tag="c_gb")
            ridx = ld(ridx_in, [128, RCH * 8], I16, tag="c_ridx")
            rdstl = ld(rdstl_in, [128, NSL], F32, tag="c_rdstl")
            rinv = ld(rinv_in, [128, NSL], F32, tag="c_rinv")
            gidx = ld(gidx_in, [128, GCH * 8], I16, tag="c_gidx")
            gdstl = ld(gdstl_in, [128, GCH], F32, tag="c_gdstl")

            adst_all = cpool.tile([128, TILES, H], F16, tag="c_adst")
            asrc_all = cpool.tile([128, TILES, H], F16, tag="c_asrc")
            eps_t = cpool.tile([128, 1], F32, tag="eps")
            nc.vector.memset(eps_t[:], LN_EPS)
            xoA = cpool.tile([128, TILES, D], F16, tag="xoA")
            xoB = cpool.tile([128, TILES, D], F16, tag="xoB")
            xo = {0: xoA, 1: xoB, 2: xoA, 3: xoB}
            nc.sync.dma_start(
                out=xoA[:],
                in_=x_own[:].rearrange("(t p) f -> p t f", p=128))

            # ---------------- RGCN layer ----------------
            def rgcn_layer(li, lnidx, src_dram, xo_cur, xo_next, ag_in, last):
                W = Ws[li]
                halves = (src_dram[0:HALF, :], src_dram[HALF:NP, :])
                st = None
                for t in range(TILES):
                    half_runs = rplan[t]
                    aggs = []
                    for hr in half_runs:
                        h, base, nch = hr["h"], hr["base"], hr["nch"]
                        aggT = psA.tile([128, R * D], F32, tag="big")
                        gt = gpool.tile([128, RWIN, D], F16, tag="rg")
                        nc.gpsimd.dma_gather(
                            gt[:, 0:nch, :], halves[h],
                            ridx[:, base * 8:(base + nch) * 8],
                            nch * 128, nch * 128, D,
                            single_packet=False)
                        for (r, j, col, st_f, sp_f) in hr["slices"]:
                            S = slpool.tile([128, 128], F16, tag="S")
                            nc.vector.tensor_scalar(
                                out=S[:], in0=iota[:],
                                scalar1=rdstl[:, col:col + 1],
                                scalar2=rinv[:, col:col + 1],
                                op0=OP.is_equal, op1=OP.mult)
                            nc.tensor.matmul(
                                aggT[:, r * D:(r + 1) * D],
                                lhsT=gt[:, j, :], rhs=S[:],
                                start=st_f, stop=sp_f)
                        agg_sb = wpool.tile([128, R * D], F16,
                                            tag=f"agg_sb{h}")
                        nc.scalar.activation(agg_sb[:, :512], aggT[:, :512],
                                             AF.Copy)
                        nc.scalar.activation(agg_sb[:, 512:], aggT[:, 512:],
                                             AF.Copy)
                        aggs.append((agg_sb, [r for (r, _j, _c, s, _p)
                                              in hr["slices"] if s]))
                    # self relation (root) via identity
                    xoT = psB.tile([128, D], F32, tag="sm")
                    nc.tensor.matmul(xoT[:], lhsT=xo_cur[:, t, :], rhs=ident[:],
                                     start=True, stop=True)
                    xoT_sb = wpool.tile([128, D], F16, tag="xoT_sb")
                    nc.scalar.activation(xoT_sb[:], xoT[:], AF.Copy)
                    outT = psB.tile([128, D], F32, tag="sm")
                    first = True
                    for (agg_sb, live) in aggs:
                        for r in live:
                            nc.tensor.matmul(outT[:],
                                             lhsT=W[:, r * D:(r + 1) * D],
                                             rhs=agg_sb[:, r * D:(r + 1) * D],
                                             start=first, stop=False)
                            first = False
                    nc.tensor.matmul(outT[:], lhsT=W[:, R * D:(R + 1) * D],
                                     rhs=xoT_sb[:], start=first, stop=True)
                    outT_sb = wpool.tile([128, D], F16, tag="outT_sb")
                    nc.scalar.activation(outT_sb[:], outT[:], AF.Copy)
                    fin = psB.tile([128, D], F16, tag="sm")
                    nc.tensor.transpose(fin[:], outT_sb[:], ident[:])
                    g = t % 4
                    if g == 0:
                        st = spool.tile([128, 4, D], F32, tag="st")
                    nc.vector.tensor_tensor(out=st[:, g, :], in0=fin[:],
                                            in1=BIAS[li][:], op=OP.add)
                    if g == 3 or t == TILES - 1:
                        epilogue(li, lnidx, st, g + 1, t - g, xo_next, ag_in, last)

            def epilogue(li, lnidx, st, ng, t0, xo_next, ag_in, last):
                stv = st[:, 0:ng, :]
                r1 = wpool.tile([128, 4], F32, tag="r1")
                nc.vector.tensor_reduce(r1[:, :ng], stv, axis=mybir.AxisListType.X,
                                        op=OP.add)
                sq = wpool.tile([128, 4, D], F32, tag="sq")
                nc.vector.tensor_tensor(out=sq[:, :ng, :], in0=stv, in1=stv,
                                        op=OP.mult)
                r2 = wpool.tile([128, 4], F32, tag="r2")
                nc.vector.tensor_reduce(r2[:, :ng], sq[:, :ng, :],
                                        axis=mybir.AxisListType.X, op=OP.add)
                if last:
                    nrm = wpool.tile([128, 4], F32, tag="nrm")
                    nc.scalar.activation(nrm[:, :ng], r2[:, :ng], AF.Sqrt)
                    nc.vector.tensor_scalar_max(nrm[:, :ng], nrm[:, :ng], 1e-12)
                    rin = wpool.tile([128, 4], F32, tag="rin")
                    nc.vector.reciprocal(rin[:, :ng], nrm[:, :ng])
                    y = wpool.tile([128, 4, D], F32, tag="y")
                    nc.vector.tensor_tensor(
                        out=y[:, :ng, :], in0=stv,
                        in1=bc(rin[:, :ng], [[1, ng], [0, D]]),
                        op=OP.mult)
                    nc.sync.dma_start(
                        out=out_dram[t0 * 128:(t0 + ng) * 128, :].rearrange(
                            "(a p) f -> p a f", p=128),
                        in_=y[:, :ng, :])
                    return
                mu = wpool.tile([128, 4], F32, tag="mu")
                nc.vector.tensor_scalar_mul(mu[:, :ng], r1[:, :ng], 1.0 / D)
                ex2 = wpool.tile([128, 4], F32, tag="ex2")
                nc.vector.tensor_scalar_mul(ex2[:, :ng], r2[:, :ng], 1.0 / D)
                mu2 = wpool.tile([128, 4], F32, tag="mu2")
                nc.vector.tensor_tensor(out=mu2[:, :ng], in0=mu[:, :ng],
                                        in1=mu[:, :ng], op=OP.mult)
                var = wpool.tile([128, 4], F32, tag="var")
                nc.vector.tensor_tensor(out=var[:, :ng], in0=ex2[:, :ng],
                                        in1=mu2[:, :ng], op=OP.subtract)
                sd = wpool.tile([128, 4], F32, tag="sd")
                nc.scalar.activation(sd[:, :ng], var[:, :ng], AF.Sqrt,
                                     bias=eps_t[:])
                rstd = wpool.tile([128, 4], F32, tag="rstd")
                nc.vector.reciprocal(rstd[:, :ng], sd[:, :ng])
                xc = wpool.tile([128, 4, D], F32, tag="xc")
                nc.vector.tensor_tensor(
                    out=xc[:, :ng, :], in0=stv,
                    in1=bc(mu[:, :ng], [[1, ng], [0, D]]),
                    op=OP.subtract)
                nc.vector.tensor_tensor(
                    out=xc[:, :ng, :], in0=xc[:, :ng, :],
                    in1=bc(rstd[:, :ng], [[1, ng], [0, D]]),
                    op=OP.mult)
                nc.vector.tensor_tensor(
                    out=xc[:, :ng, :], in0=xc[:, :ng, :],
                    in1=bc(LNg[lnidx][:], [[0, ng], [1, D]]),
                    op=OP.mult)
                nc.vector.tensor_tensor(
                    out=xc[:, :ng, :], in0=xc[:, :ng, :],
                    in1=bc(LNb[lnidx][:], [[0, ng], [1, D]]),
                    op=OP.add)
                tmp = wpool.tile([128, 4, D], F32, tag="lk")
                nc.vector.tensor_scalar_mul(tmp[:, :ng, :], xc[:, :ng, :], NEG)
                nc.vector.tensor_tensor(out=xo_next[:, t0:t0 + ng, :],
                                        in0=xc[:, :ng, :], in1=tmp[:, :ng, :],
                                        op=OP.max)
                if li == 1:
                    for tt in range(t0, t0 + ng):
                        yT = psB.tile([128, D], F16, tag="sm")
                        nc.tensor.transpose(yT[:], xo_next[:, tt, :], ident[:])
                        yT_sb = wpool.tile([128, D], F16, tag="yT_sb")
                        nc.scalar.activation(yT_sb[:], yT[:], AF.Copy)
                        alph = psB.tile([128, 2 * H], F32, tag="sm")
                        nc.tensor.matmul(alph[:], lhsT=yT_sb[:], rhs=gu_sb[:],
                                         start=True, stop=True)
                        ext = wpool.tile([128, EXTD], F16, tag="ext")
                        nc.vector.tensor_copy(ext[:, 0:D], xo_next[:, tt, :])
                        nc.scalar.activation(ext[:, D:D + 2 * H], alph[:],
                                             AF.Copy)
                        nc.sync.dma_start(
                            out=ag_in[tt * 128:(tt + 1) * 128, :], in_=ext[:])
                        nc.scalar.activation(adst_all[:, tt, :],
                                             alph[:, H:2 * H], AF.Copy)
                        nc.scalar.activation(asrc_all[:, tt, :],
                                             alph[:, 0:H], AF.Copy)
                else:
                    nc.sync.dma_start(
                        out=ag_in[t0 * 128:(t0 + ng) * 128, :].rearrange(
                            "(a p) f -> p a f", p=128),
                        in_=xo_next[:, t0:t0 + ng, :])

            # ---------------- GAT layer ----------------
            def gat_layer(xo_cur, xo_next, ag_in):
                halves = (xex2[0:HALF, :], xex2[HALF:NP, :])
                st = None
                for t in range(TILES):
                    runs = gplan[t]
                    total_ch = sum(nch for _h, _b, nch in runs)
                    agg4 = psA.tile([128, H * D], F32, tag="big")
                    den = psB.tile([128, H], F32, tag="sm")
                    firstmm = True
                    nmm = 0
                    for (h, base, nch) in runs:
                        gt = gpool.tile([128, GWIN, EXTD], F16, tag="gx")
                        nc.gpsimd.dma_gather(
                            gt[:, 0:nch, :], halves[h],
                            gidx[:, base * 8:(base + nch) * 8],
                            nch * 128, nch * 128, EXTD,
                            single_packet=False)
                        for j in range(nch):
                            cs = base + j
                            S01 = slpool.tile([128, 128], F16, tag="S01")
                            nc.vector.tensor_scalar(
                                out=S01[:], in0=iota[:],
                                scalar1=gdstl[:, cs:cs + 1],
                                scalar2=None,
                                op0=OP.is_equal)
                            # per-edge alpha_dst via S01^T @ adst_tile
                            S01T_ps = psB.tile([128, 128], F16, tag="sm")
                            nc.tensor.transpose(S01T_ps[:], S01[:], ident[:])
                            S01T_sb = wpool.tile([128, 128], F16, tag="s01t")
                            nc.scalar.activation(S01T_sb[:], S01T_ps[:],
                                                 AF.Copy)
                            adp = psB.tile([128, H], F32, tag="sm")
                            nc.tensor.matmul(adp[:], lhsT=S01T_sb[:],
                                             rhs=adst_all[:, t, :],
                                             start=True, stop=True)
                            exl = slpool.tile([128, H], F16, tag="exl")
                            nc.vector.tensor_tensor(
                                out=exl[:], in0=gt[:, j, D:D + H],
                                in1=adp[:], op=OP.add)
                            lk = slpool.tile([128, H], F16, tag="lkg")
                            nc.vector.tensor_scalar_mul(lk[:], exl[:], GAT_NEG)
                            nc.vector.tensor_tensor(out=exl[:], in0=exl[:],
                                                    in1=lk[:], op=OP.max)
                            nc.scalar.activation(exl[:], exl[:], AF.Exp)
                            xs = slpool.tile([128, H, D], F16, tag="xs")
                            nc.vector.tensor_tensor(
                                out=xs[:],
                                in0=bc(gt[:, j, 0:D], [[0, H], [1, D]]),
                                in1=bc(exl[:], [[1, H], [0, D]]),
                                op=OP.mult)
                            nmm += 1
                            lastmm = (nmm == total_ch)
                            nc.tensor.matmul(agg4[:], lhsT=S01[:],
                                             rhs=xs[:],
                                             start=firstmm, stop=lastmm)
                            nc.tensor.matmul(den[:], lhsT=S01[:],
                                             rhs=exl[:],
                                             start=firstmm, stop=lastmm)
                            firstmm = False
                    # ---- self-loop term (not gathered) ----
                    lsf = wpool.tile([128, H], F32, tag="lsf")
                    nc.vector.tensor_tensor(out=lsf[:], in0=asrc_all[:, t, :],
                                            in1=adst_all[:, t, :], op=OP.add)
                    lsk = wpool.tile([128, H], F32, tag="lsk")
                    nc.vector.tensor_scalar_mul(lsk[:], lsf[:], GAT_NEG)
                    nc.vector.tensor_tensor(out=lsf[:], in0=lsf[:],
                                            in1=lsk[:], op=OP.max)
                    exs = wpool.tile([128, H], F32, tag="exs")
                    nc.scalar.activation(exs[:], lsf[:], AF.Exp)
                    den_sb = wpool.tile([128, H], F32, tag="den_sb")
                    nc.vector.tensor_tensor(out=den_sb[:], in0=den[:],
                                            in1=exs[:], op=OP.add)
                    rden = wpool.tile([128, H], F32, tag="rden")
                    nc.vector.reciprocal(rden[:], den_sb[:])
                    rden16 = wpool.tile([128, H], F16, tag="rden16")
                    nc.vector.tensor_copy(rden16[:], rden[:])
                    agg_sb = wpool.tile([128, H * D], F16, tag="agg_sb")
                    nc.scalar.activation(agg_sb[:], agg4[:], AF.Copy)
                    for hh in range(H):
                        selfh = wpool.tile([128, D], F16, tag="selfh")
                        nc.vector.tensor_scalar(
                            out=selfh[:], in0=xo_cur[:, t, :],
                            scalar1=exs[:, hh:hh + 1], scalar2=None,
                            op0=OP.mult)
                        nc.vector.tensor_tensor(
                            out=agg_sb[:, hh * D:(hh + 1) * D],
                            in0=agg_sb[:, hh * D:(hh + 1) * D],
                            in1=selfh[:], op=OP.add)
                    aggTS = psA.tile([128, H * D], F32, tag="big")
                    Dh = wpool.tile([128, H, 128], F16, tag="Dh")
                    nc.vector.tensor_tensor(
                        out=Dh[:],
                        in0=bc(ident[:], [[0, H], [1, 128]]),
                        in1=bc(rden16[:], [[1, H], [0, 128]]),
                        op=OP.mult)
                    for hh in range(H):
                        nc.tensor.matmul(aggTS[:, hh * D:(hh + 1) * D],
                                         lhsT=agg_sb[:, hh * D:(hh + 1) * D],
                                         rhs=Dh[:, hh, :], start=True, stop=True)
                    aggTS_sb = wpool.tile([128, H * D], F16, tag="aggTS_sb")
                    nc.scalar.activation(aggTS_sb[:], aggTS[:], AF.Copy)
                    outT = psB.tile([128, D], F32, tag="sm")
                    for hh in range(H):
                        nc.tensor.matmul(outT[:], lhsT=gw_sb[:, hh * D:(hh + 1) * D],
                                         rhs=aggTS_sb[:, hh * D:(hh + 1) * D],
                                         start=(hh == 0), stop=(hh == H - 1))
                    outT_sb = wpool.tile([128, D], F16, tag="outT_sb")
                    nc.scalar.activation(outT_sb[:], outT[:], AF.Copy)
                    fin = psB.tile([128, D], F16, tag="sm")
                    nc.tensor.transpose(fin[:], outT_sb[:], ident[:])
                    g = t % 4
                    if g == 0:
                        st = spool.tile([128, 4, D], F32, tag="st")
                    nc.vector.tensor_tensor(out=st[:, g, :], in0=fin[:],
                                            in1=gb_sb[:], op=OP.add)
                    if g == 3 or t == TILES - 1:
                        epilogue(2, 2, st, g + 1, t - g, xo_next, ag_in, False)

            def exchange(ag_in_ap, xex_ap):
                nc.gpsimd.collective_compute(
                    "AllGather", OP.bypass, replica_groups=rg,
                    ins=[ag_in_ap[:]], outs=[xex_ap[:]])

            # ---------------- program ----------------
            import os
            KREP = int(os.environ.get("KREPEAT", "1"))
            for _rep in range(KREP):
                if _rep > 0:
                    nc.sync.dma_start(
                        out=xoA[:],
                        in_=x_own[:].rearrange("(t p) f -> p t f", p=128))
                nc.sync.dma_start(out=xex0[:], in_=x_pad[:])
                rgcn_layer(0, 0, xex0, xo[0], xo[1], ag0_in, False)
                exchange(ag0_in, xex1)
                rgcn_layer(1, 1, xex1, xo[1], xo[2], ag1_in, False)
                exchange(ag1_in, xex2)
                gat_layer(xo[2], xo[3], ag2_in)
                exchange(ag2_in, xex3)
                rgcn_layer(3, None, xex3, xo[3], None, None, True)

    nc.compile()
    return nc


# ----------------------------------------------------------------------------
# Public API
# ----------------------------------------------------------------------------

_CACHE = {}


def kernel(**inputs):
    cfg = default_cfg()
    N, CORES = cfg["N"], cfg["CORES"]

    key = "k"
    edge_index = np.asarray(inputs["edge_index"])
    edge_type = np.asarray(inputs["edge_type"])
    if key not in _CACHE:
        plan, per_core = build_graph_plan(cfg, edge_index, edge_type)
        nc = build_nc(cfg, plan)
        _CACHE[key] = (nc, plan, per_core)
    nc, plan, per_core = _CACHE[key]

    in_maps = build_in_maps(cfg, inputs, per_core)
    res = run_bass_kernel_spmd(nc, in_maps, list(range(CORES)))
    out = np.concatenate([res.results[c]["out"] for c in range(CORES)], axis=0)
    return out[:N].astype(np.float32)


# revision 9
# speedup vs baseline: 1.5616x; 1.5616x over previous
"""AttentionRGCN (3x RGCN + GAT) Trainium2 Bass kernel, 8-core SPMD.

Strategy: shard nodes (dst) across 8 cores; edges live with their dst core.
Per dst-tile (128 nodes) aggregation via one-hot matmuls:
  aggT[f, d] (per relation) = sum_e x[src_e, f] * S[e, d],
  S[e, d] = (d == dst_local[e]) * inv_deg[e]
One fused DVE tensor_scalar builds each S slice:
  S = (iota is_equal dstl_col) mult inv_col     (per-partition scalars)
Edges are packed per-core CONTIGUOUSLY per (tile, half) run (sorted by rel),
padded to the cross-core max rounded to 128; relation boundaries drift per
core, so each (chunk, rel) overlap gets its own sentinel-masked S slice.
Per-edge source rows fetched with gpsimd.dma_gather (fp16 256B rows, int16
idx, half-split tables for the 32k index range), one gather per run.
Between layers: AllGather. GAT: self-loops are NOT gathered (self term is
computed directly from local features); attention logits from gathered
[x | alpha_src] ext rows + local alpha_dst via transposed one-hot matmul;
segment softmax without max-subtraction; denominator applied per-dst AFTER
aggregation via a diagonal matmul.

Data path is fp16; accumulation (PSUM), LayerNorm statistics and epilogue
math stay fp32. PSUM->SBUF copies ride the idle Scalar (ACT) engine.
"""
import sys
sys.path.insert(0, "/opt/trn_rl_repo")
import numpy as np

import concourse.bass as bass
import concourse.bacc as bacc
import concourse.mybir as mybir
import concourse.tile as tile
from concourse.bass_utils import run_bass_kernel_spmd


def bc(ap_obj, dims):
    """Custom broadcast AP: keep partition dim of ap_obj, replace free dims."""
    return bass.AP(ap_obj.tensor, ap_obj.offset, [list(ap_obj.ap[0])] + dims)

F32 = mybir.dt.float32
F16 = mybir.dt.float16
I16 = mybir.dt.int16
AF = mybir.ActivationFunctionType
OP = mybir.AluOpType

NEG = 0.1
LN_EPS = 1e-5
GAT_NEG = 0.2
SENT = 999.0


def default_cfg():
    return dict(N=50000, NP=50176, E=600000, R=8, B=8, D=128, H=4,
                CORES=8, PER=6272, TILES=49, HALF=25088, EXTD=256)


# ----------------------------------------------------------------------------
# Host-side graph preprocessing
# ----------------------------------------------------------------------------

def wrap_idx(flat: np.ndarray) -> np.ndarray:
    """int16 flat idx list (len mult of 128) -> [128, len/16] wrapped layout."""
    n = len(flat)
    assert n % 128 == 0
    w = flat.astype(np.int16).reshape(n // 16, 16).T  # [16, n/16]
    return np.tile(w, (8, 1))


def build_graph_plan(cfg, edge_index, edge_type):
    """Per-core contiguous packing with shared run widths and union slices.

    plan:
      rplan: per tile -> list of per-half dicts:
          (h, chunk_base, nch, slices=[(rel, j, col, start, stop), ...])
      gplan: per tile -> list of (h, chunk_base, nch)
      r_total_ch, g_total_ch, r_total_sl, rwin, gwin
    per_core[c]: ridx, rdstl, rinv (per-slice cols), gidx, gdstl
    """
    N, NP, R = cfg["N"], cfg["NP"], cfg["R"]
    CORES, PER, TILES, HALF = cfg["CORES"], cfg["PER"], cfg["TILES"], cfg["HALF"]
    src, dst = edge_index[0].astype(np.int64), edge_index[1].astype(np.int64)
    rel = edge_type.astype(np.int64)

    deg = np.bincount(rel * N + dst, minlength=R * N).astype(np.float32)
    inv_tab = np.float32(1.0) / np.maximum(deg, np.float32(1.0))

    core_of = dst // PER
    NRUN = TILES * 2

    # ---- per-core sorted edge arrays ----
    r_edges = []  # per core: (runkey, rel, src, dstl128, inv) sorted
    g_edges = []  # per core: (runkey, src, dstl128) sorted (no self loops)
    for c in range(CORES):
        m = core_of == c
        s_c, d_c, r_c = src[m], dst[m], rel[m]
        dl = d_c - c * PER
        t_c = dl // 128
        h_c = (s_c >= HALF).astype(np.int64)
        runk = t_c * 2 + h_c
        order = np.lexsort((r_c, runk))
        r_edges.append((runk[order], r_c[order], s_c[order],
                        (d_c[order] % 128).astype(np.float32),
                        inv_tab[r_c[order] * N + d_c[order]]))
        gorder = np.argsort(runk, kind="stable")
        g_edges.append((runk[gorder], s_c[gorder],
                        (d_c[gorder] % 128).astype(np.float32)))

    # ---- run widths ----
    rcnt = np.zeros((CORES, NRUN), np.int64)   # edges per (core, run)
    rcnt_cr = np.zeros((CORES, NRUN, R), np.int64)
    gcnt = np.zeros((CORES, NRUN), np.int64)
    for c in range(CORES):
        runk, rl = r_edges[c][0], r_edges[c][1]
        np.add.at(rcnt[c], runk, 1)
        np.add.at(rcnt_cr[c], (runk, rl), 1)
        np.add.at(gcnt[c], g_edges[c][0], 1)
    rW = (np.ceil(rcnt.max(axis=0) / 128).astype(np.int64)) * 128
    gW = (np.ceil(gcnt.max(axis=0) / 128).astype(np.int64)) * 128
    # per-core start offsets of each rel within its run (packed)
    rstart = np.cumsum(rcnt_cr, axis=2) - rcnt_cr  # [C, NRUN, R]
    rend = rstart + rcnt_cr

    # ---- chunk bases + slices ----
    rplan = [[] for _ in range(TILES)]
    gplan = [[] for _ in range(TILES)]
    r_total_ch = 0
    g_total_ch = 0
    r_total_sl = 0
    rsl_info = []   # (run, rel, j, col) in emit order
    rch_base = np.zeros(NRUN, np.int64)
    gch_base = np.zeros(NRUN, np.int64)
    for run in range(NRUN):
        t, h = run // 2, run % 2
        nch = int(rW[run] // 128)
        rch_base[run] = r_total_ch
        slices = []
        if nch:
            for r in range(R):
                act = rcnt_cr[:, run, r] > 0
                if not act.any():
                    continue
                jlo = int((rstart[act, run, r] // 128).min())
                jhi = int(((rend[act, run, r] - 1) // 128).max()) + 1
                js = list(range(jlo, jhi))
                for k, j in enumerate(js):
                    col = r_total_sl
                    r_total_sl += 1
                    slices.append((r, j, col, k == 0, k == len(js) - 1))
                    rsl_info.append((run, r, j, col))
            rplan[t].append(dict(h=h, base=r_total_ch, nch=nch,
                                 slices=slices))
            r_total_ch += nch
        gn = int(gW[run] // 128)
        gch_base[run] = g_total_ch
        if gn:
            gplan[t].append((h, g_total_ch, gn))
            g_total_ch += gn
    rwin = max((p["nch"] for tp in rplan for p in tp), default=1)
    gwin = max((g[2] for tp in gplan for g in tp), default=1)
    rsl_max = max((len(p["slices"]) for tp in rplan for p in tp), default=1)

    # ---- per-core tables ----
    per_core = []
    for c in range(CORES):
        runk, rl, s_c, dl_c, inv_c = r_edges[c]
        rbounds = np.searchsorted(runk, np.arange(NRUN + 1))
        ridx = np.zeros(r_total_ch * 128, np.int64)
        # per-run packed values
        run_dstl = np.full(r_total_ch * 128, SENT, np.float32)
        run_inv = np.zeros(r_total_ch * 128, np.float32)
        run_rel = np.full(r_total_ch * 128, -1, np.int64)
        for run in range(NRUN):
            lo, hi = rbounds[run], rbounds[run + 1]
            cnt = hi - lo
            if rW[run] == 0:
                continue
            p0 = rch_base[run] * 128
            ridx[p0:p0 + cnt] = s_c[lo:hi] - (run % 2) * HALF
            run_dstl[p0:p0 + cnt] = dl_c[lo:hi]
            run_inv[p0:p0 + cnt] = inv_c[lo:hi]
            run_rel[p0:p0 + cnt] = rl[lo:hi]
        # per-slice columns
        rdstl = np.full((r_total_sl, 128), SENT, np.float32)
        rinv = np.zeros((r_total_sl, 128), np.float32)
        for (run, r, j, col) in rsl_info:
            p0 = (rch_base[run] + j) * 128
            seg = slice(p0, p0 + 128)
            msk = run_rel[seg] == r
            rdstl[col][msk] = run_dstl[seg][msk]
            rinv[col][msk] = run_inv[seg][msk]

        grunk, gs_c, gdl_c = g_edges[c]
        gbounds = np.searchsorted(grunk, np.arange(NRUN + 1))
        gidx = np.zeros(g_total_ch * 128, np.int64)
        gdstl = np.full(g_total_ch * 128, SENT, np.float32)
        for run in range(NRUN):
            lo, hi = gbounds[run], gbounds[run + 1]
            cnt = hi - lo
            if gW[run] == 0:
                continue
            p0 = gch_base[run] * 128
            gidx[p0:p0 + cnt] = gs_c[lo:hi] - (run % 2) * HALF
            gdstl[p0:p0 + cnt] = gdl_c[lo:hi]

        # dense streamed S tables: one [128,128] fp16 block per slice/chunk
        rS = np.zeros((128, r_total_sl * 128), np.float16)
        pp, cc = np.nonzero(rdstl < 128)          # [NSLICE,128] valid slots
        rS[cc, pp * 128 + rdstl[pp, cc].astype(np.int64)] = rinv[pp, cc]
        g2 = gdstl.reshape(g_total_ch, 128)
        gS = np.zeros((128, g_total_ch * 128), np.float16)
        gp_, gc_ = np.nonzero(g2 < 128)
        gS[gc_, gp_ * 128 + g2[gp_, gc_].astype(np.int64)] = 1.0
        gST = np.zeros((128, g_total_ch * 128), np.float16)
        gST[g2[gp_, gc_].astype(np.int64), gp_ * 128 + gc_] = 1.0
        per_core.append(dict(
            ridx=wrap_idx(ridx),
            rs=rS,
            gidx=wrap_idx(gidx),
            gs=gS,
            gst=gST,
        ))

    plan = dict(rplan=rplan, gplan=gplan, r_total_ch=r_total_ch,
                g_total_ch=g_total_ch, r_total_sl=r_total_sl,
                rwin=rwin, gwin=gwin, rsl_max=rsl_max)
    return plan, per_core


# ----------------------------------------------------------------------------
# Weight preprocessing (host)
# ----------------------------------------------------------------------------

def prep_weights(cfg, inp):
    D, H = cfg["D"], cfg["H"]
    out = {}
    for li, pre in (("0", "r0"), ("1", "r1"), ("3", "r2")):
        W = np.einsum("rb,bio->rio", inp[pre + "_comp"], inp[pre + "_basis"])
        Wstack = np.concatenate([W[r] for r in range(cfg["R"])] +
                                [inp[pre + "_root"]], axis=1)  # [D, 9*D]
        out["w" + li] = Wstack.astype(np.float16)
        out["bias" + li] = np.tile(inp[pre + "_bias"][None, :], (128, 1)).astype(np.float32)
    gw = inp["gat_w"]  # [D, H*D]
    out["gatw"] = (gw / H).astype(np.float16)
    U = np.zeros((D, 2 * H), np.float32)
    for h in range(H):
        Wh = gw[:, h * D:(h + 1) * D]
        U[:, h] = Wh @ inp["gat_asrc"][h]
        U[:, H + h] = Wh @ inp["gat_adst"][h]
    out["gatu"] = U.astype(np.float16)
    out["gbias"] = np.tile(inp["gat_bias"][None, :], (128, 1)).astype(np.float32)
    for k in ("ln0", "ln1", "ln2"):
        out[k + "g"] = np.tile(inp[k + "_g"][None, :], (128, 1)).astype(np.float32)
        out[k + "b"] = np.tile(inp[k + "_b"][None, :], (128, 1)).astype(np.float32)
    ident = np.zeros((128, 128), np.float16)
    np.fill_diagonal(ident, 1.0)
    out["ident"] = ident
    return out


def build_in_maps(cfg, inputs, per_core):
    N, NP, CORES, PER = cfg["N"], cfg["NP"], cfg["CORES"], cfg["PER"]
    wts = prep_weights(cfg, inputs)
    x = np.asarray(inputs["x"], dtype=np.float32)
    x_pad = np.zeros((NP, cfg["D"]), np.float16)
    x_pad[:N] = x.astype(np.float16)
    in_maps = []
    for c in range(CORES):
        m = dict(wts)
        m["x_pad"] = x_pad
        m["x_own"] = x_pad[c * PER:(c + 1) * PER]
        m.update(per_core[c])
        in_maps.append(m)
    return in_maps


# ----------------------------------------------------------------------------
# Bass program
# ----------------------------------------------------------------------------

def build_nc(cfg, plan):
    N, NP, R, D, H = cfg["N"], cfg["NP"], cfg["R"], cfg["D"], cfg["H"]
    CORES, PER, TILES, HALF = cfg["CORES"], cfg["PER"], cfg["TILES"], cfg["HALF"]
    RCH, GCH = plan["r_total_ch"], plan["g_total_ch"]
    NSL = plan["r_total_sl"]
    rplan, gplan = plan["rplan"], plan["gplan"]
    EXTD = cfg["EXTD"]  # ext row fp16 elems: [x(128) | a(8) | pad]
    RWIN, GWIN = plan["rwin"], plan["gwin"]
    RSL_MAX = plan["rsl_max"]

    nc = bacc.Bacc("TRN2", target_bir_lowering=False, debug=False,
                   num_devices=CORES)

    def inp(name, shape, dt=F16):
        return nc.dram_tensor(name, shape, dt, kind="ExternalInput").ap()

    x_pad = inp("x_pad", [NP, D])
    x_own = inp("x_own", [PER, D])
    w0, w1, w3 = (inp(k, [D, (R + 1) * D]) for k in ("w0", "w1", "w3"))
    bias0, bias1, bias3 = (inp(k, [128, D], F32) for k in ("bias0", "bias1", "bias3"))
    gatw = inp("gatw", [D, H * D])
    gatu = inp("gatu", [D, 2 * H])
    gbias = inp("gbias", [128, D], F32)
    ln0g, ln0b = inp("ln0g", [128, D], F32), inp("ln0b", [128, D], F32)
    ln1g, ln1b = inp("ln1g", [128, D], F32), inp("ln1b", [128, D], F32)
    ln2g, ln2b = inp("ln2g", [128, D], F32), inp("ln2b", [128, D], F32)
    ident_in = inp("ident", [128, 128])
    ridx_in = inp("ridx", [128, RCH * 8], I16)
    rs_in = inp("rs", [128, NSL * 128])
    gidx_in = inp("gidx", [128, GCH * 8], I16)
    gs_in = inp("gs", [128, GCH * 128])
    gst_in = inp("gst", [128, GCH * 128])

    out_dram = nc.dram_tensor("out", [PER, D], F32, kind="ExternalOutput").ap()

    xex0 = nc.dram_tensor("xex0", [NP, D], F16).ap()
    ag0_in = nc.dram_tensor("ag0_in", [PER, D], F16).ap()
    xex1 = nc.dram_tensor("xex1", [NP, D], F16, addr_space="Shared").ap()
    ag1_in = nc.dram_tensor("ag1_in", [PER, EXTD], F16).ap()
    xex2 = nc.dram_tensor("xex2", [NP, EXTD], F16, addr_space="Shared").ap()
    ag2_in = nc.dram_tensor("ag2_in", [PER, D], F16).ap()
    xex3 = nc.dram_tensor("xex3", [NP, D], F16, addr_space="Shared").ap()

    rg = [list(range(CORES))]

    with tile.TileContext(nc) as tc:
        with (
            tc.tile_pool(name="const", bufs=1) as cpool,
            tc.tile_pool(name="gath", bufs=2) as gpool,
            tc.tile_pool(name="work", bufs=2) as wpool,
            tc.tile_pool(name="slc", bufs=6) as slpool,
            tc.tile_pool(name="stage", bufs=2) as spool,
            tc.tile_pool(name="psA", bufs=2, space="PSUM") as psA,
            tc.tile_pool(name="psB", bufs=4, space="PSUM") as psB,
        ):
            # ---- load constants ----
            def ld(ap_in, shape, dt=F16, tag=None):
                t = cpool.tile(shape, dt, tag=tag)
                nc.sync.dma_start(out=t[:], in_=ap_in[:])
                return t

            ident = ld(ident_in, [128, 128], tag="c_ident")
            Ws = {0: ld(w0, [D, (R + 1) * D], tag="c_w0"),
                  1: ld(w1, [D, (R + 1) * D], tag="c_w1"),
                  3: ld(w3, [D, (R + 1) * D], tag="c_w3")}
            LNg = {0: ld(ln0g, [128, D], F32, tag="c_l0g"),
                   1: ld(ln1g, [128, D], F32, tag="c_l1g"),
                   2: ld(ln2g, [128, D], F32, tag="c_l2g")}
            LNb = {0: ld(ln0b, [128, D], F32, tag="c_l0b"),
                   1: ld(ln1b, [128, D], F32, tag="c_l1b"),
                   2: ld(ln2b, [128, D], F32, tag="c_l2b")}
            BIAS = {0: ld(bias0, [128, D], F32, tag="c_b0"),
                    1: ld(bias1, [128, D], F32, tag="c_b1"),
                    3: ld(bias3, [128, D], F32, tag="c_b3")}
            gw_sb = ld(gatw, [D, H * D], tag="c_gw")
            gu_sb = ld(gatu, [D, 2 * H], tag="c_gu")
            gb_sb = ld(gbias, [128, D], F32, tag="c_gb")
            ridx = ld(ridx_in, [128, RCH * 8], I16, tag="c_ridx")
            gidx = ld(gidx_in, [128, GCH * 8], I16, tag="c_gidx")

            adst_all = cpool.tile([128, TILES, H], F16, tag="c_adst")
            asrc_all = cpool.tile([128, TILES, H], F16, tag="c_asrc")
            eps_t = cpool.tile([128, 1], F32, tag="eps")
            nc.vector.memset(eps_t[:], LN_EPS)
            xoA = cpool.tile([128, TILES, D], F16, tag="xoA")
            xoB = cpool.tile([128, TILES, D], F16, tag="xoB")
            xo = {0: xoA, 1: xoB, 2: xoA, 3: xoB}
            nc.sync.dma_start(
                out=xoA[:],
                in_=x_own[:].rearrange("(t p) f -> p t f", p=128))

            # ---------------- RGCN layer ----------------
            def rgcn_layer(li, lnidx, src_dram, xo_cur, xo_next, ag_in, last):
                W = Ws[li]
                halves = (src_dram[0:HALF, :], src_dram[HALF:NP, :])
                st = None
                for t in range(TILES):
                    half_runs = rplan[t]
                    aggs = []
                    for hr in half_runs:
                        h, base, nch = hr["h"], hr["base"], hr["nch"]
                        slices = hr["slices"]
                        c0 = slices[0][2]
                        nsl = len(slices)
                        aggT = psA.tile([128, R * D], F32, tag="big")
                        gt = gpool.tile([128, RWIN, D], F16, tag="rg")
                        nc.gpsimd.dma_gather(
                            gt[:, 0:nch, :], halves[h],
                            ridx[:, base * 8:(base + nch) * 8],
                            nch * 128, nch * 128, D,
                            single_packet=False)
                        Sst = gpool.tile([128, RSL_MAX, 128], F16, tag="Sst")
                        nc.scalar.dma_start(
                            out=Sst[:, 0:nsl, :],
                            in_=rs_in[:, c0 * 128:(c0 + nsl) * 128].rearrange(
                                "p (s d) -> p s d", d=128))
                        for si, (r, j, col, st_f, sp_f) in enumerate(slices):
                            nc.tensor.matmul(
                                aggT[:, r * D:(r + 1) * D],
                                lhsT=gt[:, j, :], rhs=Sst[:, si, :],
                                start=st_f, stop=sp_f)
                        agg_sb = wpool.tile([128, R * D], F16,
                                            tag=f"agg_sb{h}")
                        nc.scalar.activation(agg_sb[:, :512], aggT[:, :512],
                                             AF.Copy)
                        nc.scalar.activation(agg_sb[:, 512:], aggT[:, 512:],
                                             AF.Copy)
                        aggs.append((agg_sb, [r for (r, _j, _c, s, _p)
                                              in hr["slices"] if s]))
                    # self relation (root) via identity
                    xoT = psB.tile([128, D], F32, tag="sm")
                    nc.tensor.matmul(xoT[:], lhsT=xo_cur[:, t, :], rhs=ident[:],
                                     start=True, stop=True)
                    xoT_sb = wpool.tile([128, D], F16, tag="xoT_sb")
                    nc.scalar.activation(xoT_sb[:], xoT[:], AF.Copy)
                    outT = psB.tile([128, D], F32, tag="sm")
                    first = True
                    for (agg_sb, live) in aggs:
                        for r in live:
                            nc.tensor.matmul(outT[:],
                                             lhsT=W[:, r * D:(r + 1) * D],
                                             rhs=agg_sb[:, r * D:(r + 1) * D],
                                             start=first, stop=False)
                            first = False
                    nc.tensor.matmul(outT[:], lhsT=W[:, R * D:(R + 1) * D],
                                     rhs=xoT_sb[:], start=first, stop=True)
                    outT_sb = wpool.tile([128, D], F16, tag="outT_sb")
                    nc.scalar.activation(outT_sb[:], outT[:], AF.Copy)
                    fin = psB.tile([128, D], F16, tag="sm")
                    nc.tensor.transpose(fin[:], outT_sb[:], ident[:])
                    g = t % 4
                    if g == 0:
                        st = spool.tile([128, 4, D], F32, tag="st")
                    nc.vector.tensor_tensor(out=st[:, g, :], in0=fin[:],
                                            in1=BIAS[li][:], op=OP.add)
                    if g == 3 or t == TILES - 1:
                        epilogue(li, lnidx, st, g + 1, t - g, xo_next, ag_in, last)

            def epilogue(li, lnidx, st, ng, t0, xo_next, ag_in, last):
                stv = st[:, 0:ng, :]
                r1 = wpool.tile([128, 4], F32, tag="r1")
                nc.vector.tensor_reduce(r1[:, :ng], stv, axis=mybir.AxisListType.X,
                                        op=OP.add)
                sq = wpool.tile([128, 4, D], F32, tag="sq")
                nc.vector.tensor_tensor(out=sq[:, :ng, :], in0=stv, in1=stv,
                                        op=OP.mult)
                r2 = wpool.tile([128, 4], F32, tag="r2")
                nc.vector.tensor_reduce(r2[:, :ng], sq[:, :ng, :],
                                        axis=mybir.AxisListType.X, op=OP.add)
                if last:
                    nrm = wpool.tile([128, 4], F32, tag="nrm")
                    nc.scalar.activation(nrm[:, :ng], r2[:, :ng], AF.Sqrt)
                    nc.vector.tensor_scalar_max(nrm[:, :ng], nrm[:, :ng], 1e-12)
                    rin = wpool.tile([128, 4], F32, tag="rin")
                    nc.vector.reciprocal(rin[:, :ng], nrm[:, :ng])
                    y = wpool.tile([128, 4, D], F32, tag="y")
                    nc.vector.tensor_tensor(
                        out=y[:, :ng, :], in0=stv,
                        in1=bc(rin[:, :ng], [[1, ng], [0, D]]),
                        op=OP.mult)
                    nc.sync.dma_start(
                        out=out_dram[t0 * 128:(t0 + ng) * 128, :].rearrange(
                            "(a p) f -> p a f", p=128),
                        in_=y[:, :ng, :])
                    return
                mu = wpool.tile([128, 4], F32, tag="mu")
                nc.vector.tensor_scalar_mul(mu[:, :ng], r1[:, :ng], 1.0 / D)
                ex2 = wpool.tile([128, 4], F32, tag="ex2")
                nc.vector.tensor_scalar_mul(ex2[:, :ng], r2[:, :ng], 1.0 / D)
                mu2 = wpool.tile([128, 4], F32, tag="mu2")
                nc.vector.tensor_tensor(out=mu2[:, :ng], in0=mu[:, :ng],
                                        in1=mu[:, :ng], op=OP.mult)
                var = wpool.tile([128, 4], F32, tag="var")
                nc.vector.tensor_tensor(out=var[:, :ng], in0=ex2[:, :ng],
                                        in1=mu2[:, :ng], op=OP.subtract)
                sd = wpool.tile([128, 4], F32, tag="sd")
                nc.scalar.activation(sd[:, :ng], var[:, :ng], AF.Sqrt,
                                     bias=eps_t[:])
                rstd = wpool.tile([128, 4], F32, tag="rstd")
                nc.vector.reciprocal(rstd[:, :ng], sd[:, :ng])
                xc = wpool.tile([128, 4, D], F32, tag="xc")
                nc.vector.tensor_tensor(
                    out=xc[:, :ng, :], in0=stv,
                    in1=bc(mu[:, :ng], [[1, ng], [0, D]]),
                    op=OP.subtract)
                nc.vector.tensor_tensor(
                    out=xc[:, :ng, :], in0=xc[:, :ng, :],
                    in1=bc(rstd[:, :ng], [[1, ng], [0, D]]),
                    op=OP.mult)
                nc.vector.tensor_tensor(
                    out=xc[:, :ng, :], in0=xc[:, :ng, :],
                    in1=bc(LNg[lnidx][:], [[0, ng], [1, D]]),
                    op=OP.mult)
                nc.vector.tensor_tensor(
                    out=xc[:, :ng, :], in0=xc[:, :ng, :],
                    in1=bc(LNb[lnidx][:], [[0, ng], [1, D]]),
                    op=OP.add)
                tmp = wpool.tile([128, 4, D], F32, tag="lk")
                nc.vector.tensor_scalar_mul(tmp[:, :ng, :], xc[:, :ng, :], NEG)
                nc.vector.tensor_tensor(out=xo_next[:, t0:t0 + ng, :],
                                        in0=xc[:, :ng, :], in1=tmp[:, :ng, :],
                                        op=OP.max)
                if li == 1:
                    for tt in range(t0, t0 + ng):
                        yT = psB.tile([128, D], F16, tag="sm")
                        nc.tensor.transpose(yT[:], xo_next[:, tt, :], ident[:])
                        yT_sb = wpool.tile([128, D], F16, tag="yT_sb")
                        nc.scalar.activation(yT_sb[:], yT[:], AF.Copy)
                        alph = psB.tile([128, 2 * H], F32, tag="sm")
                        nc.tensor.matmul(alph[:], lhsT=yT_sb[:], rhs=gu_sb[:],
                                         start=True, stop=True)
                        ext = wpool.tile([128, EXTD], F16, tag="ext")
                        nc.vector.tensor_copy(ext[:, 0:D], xo_next[:, tt, :])
                        nc.scalar.activation(ext[:, D:D + 2 * H], alph[:],
                                             AF.Copy)
                        nc.sync.dma_start(
                            out=ag_in[tt * 128:(tt + 1) * 128, :], in_=ext[:])
                        nc.scalar.activation(adst_all[:, tt, :],
                                             alph[:, H:2 * H], AF.Copy)
                        nc.scalar.activation(asrc_all[:, tt, :],
                                             alph[:, 0:H], AF.Copy)
                else:
                    nc.sync.dma_start(
                        out=ag_in[t0 * 128:(t0 + ng) * 128, :].rearrange(
                            "(a p) f -> p a f", p=128),
                        in_=xo_next[:, t0:t0 + ng, :])

            # ---------------- GAT layer ----------------
            def gat_layer(xo_cur, xo_next, ag_in):
                halves = (xex2[0:HALF, :], xex2[HALF:NP, :])
                st = None
                for t in range(TILES):
                    runs = gplan[t]
                    total_ch = sum(nch for _h, _b, nch in runs)
                    agg4 = psA.tile([128, H * D], F32, tag="big")
                    den = psB.tile([128, H], F32, tag="sm")
                    firstmm = True
                    nmm = 0
                    for (h, base, nch) in runs:
                        gt = gpool.tile([128, GWIN, EXTD], F16, tag="gx")
                        nc.gpsimd.dma_gather(
                            gt[:, 0:nch, :], halves[h],
                            gidx[:, base * 8:(base + nch) * 8],
                            nch * 128, nch * 128, EXTD,
                            single_packet=False)
                        S01 = gpool.tile([128, GWIN, 128], F16, tag="S01")
                        nc.scalar.dma_start(
                            out=S01[:, 0:nch, :],
                            in_=gs_in[:, base * 128:(base + nch) * 128].rearrange(
                                "p (s d) -> p s d", d=128))
                        S01T = gpool.tile([128, GWIN, 128], F16, tag="S01T")
                        nc.scalar.dma_start(
                            out=S01T[:, 0:nch, :],
                            in_=gst_in[:, base * 128:(base + nch) * 128].rearrange(
                                "p (s d) -> p s d", d=128))
                        for j in range(nch):
                            adp = psB.tile([128, H], F32, tag="sm")
                            nc.tensor.matmul(adp[:], lhsT=S01T[:, j, :],
                                             rhs=adst_all[:, t, :],
                                             start=True, stop=True)
                            exl = slpool.tile([128, H], F16, tag="exl")
                            nc.vector.tensor_tensor(
                                out=exl[:], in0=gt[:, j, D:D + H],
                                in1=adp[:], op=OP.add)
                            lk = slpool.tile([128, H], F16, tag="lkg")
                            nc.vector.tensor_scalar_mul(lk[:], exl[:], GAT_NEG)
                            nc.vector.tensor_tensor(out=exl[:], in0=exl[:],
                                                    in1=lk[:], op=OP.max)
                            nc.scalar.activation(exl[:], exl[:], AF.Exp)
                            xs = slpool.tile([128, H, D], F16, tag="xs")
                            nc.vector.tensor_tensor(
                                out=xs[:],
                                in0=bc(gt[:, j, 0:D], [[0, H], [1, D]]),
                                in1=bc(exl[:], [[1, H], [0, D]]),
                                op=OP.mult)
                            nmm += 1
                            lastmm = (nmm == total_ch)
                            nc.tensor.matmul(agg4[:], lhsT=S01[:, j, :],
                                             rhs=xs[:],
                                             start=firstmm, stop=lastmm)
                            nc.tensor.matmul(den[:], lhsT=S01[:, j, :],
                                             rhs=exl[:],
                                             start=firstmm, stop=lastmm)
                            firstmm = False
                    # ---- self-loop term (not gathered) ----
                    lsf = wpool.tile([128, H], F32, tag="lsf")
                    nc.vector.tensor_tensor(out=lsf[:], in0=asrc_all[:, t, :],
                                            in1=adst_all[:, t, :], op=OP.add)
                    lsk = wpool.tile([128, H], F32, tag="lsk")
                    nc.vector.tensor_scalar_mul(lsk[:], lsf[:], GAT_NEG)
                    nc.vector.tensor_tensor(out=lsf[:], in0=lsf[:],
                                            in1=lsk[:], op=OP.max)
                    exs = wpool.tile([128, H], F32, tag="exs")
                    nc.scalar.activation(exs[:], lsf[:], AF.Exp)
                    den_sb = wpool.tile([128, H], F32, tag="den_sb")
                    nc.vector.tensor_tensor(out=den_sb[:], in0=den[:],
                                            in1=exs[:], op=OP.add)
                    rden = wpool.tile([128, H], F32, tag="rden")
                    nc.vector.reciprocal(rden[:], den_sb[:])
                    agg_sb = wpool.tile([128, H * D], F16, tag="agg_sb")
                    nc.scalar.activation(agg_sb[:], agg4[:], AF.Copy)
                    for hh in range(H):
                        selfh = wpool.tile([128, D], F16, tag="selfh")
                        nc.vector.tensor_scalar(
                            out=selfh[:], in0=xo_cur[:, t, :],
                            scalar1=exs[:, hh:hh + 1], scalar2=None,
                            op0=OP.mult)
                        nc.vector.tensor_tensor(
                            out=agg_sb[:, hh * D:(hh + 1) * D],
                            in0=agg_sb[:, hh * D:(hh + 1) * D],
                            in1=selfh[:], op=OP.add)
                    aggTS = psA.tile([128, H * D], F32, tag="big")
                    for hh in range(H):
                        nc.vector.tensor_scalar(
                            out=agg_sb[:, hh * D:(hh + 1) * D],
                            in0=agg_sb[:, hh * D:(hh + 1) * D],
                            scalar1=rden[:, hh:hh + 1], scalar2=None,
                            op0=OP.mult)
                        nc.tensor.matmul(aggTS[:, hh * D:(hh + 1) * D],
                                         lhsT=agg_sb[:, hh * D:(hh + 1) * D],
                                         rhs=ident[:], start=True, stop=True)
                    aggTS_sb = wpool.tile([128, H * D], F16, tag="aggTS_sb")
                    nc.scalar.activation(aggTS_sb[:], aggTS[:], AF.Copy)
                    outT = psB.tile([128, D], F32, tag="sm")
                    for hh in range(H):
                        nc.tensor.matmul(outT[:], lhsT=gw_sb[:, hh * D:(hh + 1) * D],
                                         rhs=aggTS_sb[:, hh * D:(hh + 1) * D],
                                         start=(hh == 0), stop=(hh == H - 1))
                    outT_sb = wpool.tile([128, D], F16, tag="outT_sb")
                    nc.scalar.activation(outT_sb[:], outT[:], AF.Copy)
                    fin = psB.tile([128, D], F16, tag="sm")
                    nc.tensor.transpose(fin[:], outT_sb[:], ident[:])
                    g = t % 4
                    if g == 0:
                        st = spool.tile([128, 4, D], F32, tag="st")
                    nc.vector.tensor_tensor(out=st[:, g, :], in0=fin[:],
                                            in1=gb_sb[:], op=OP.add)
                    if g == 3 or t == TILES - 1:
                        epilogue(2, 2, st, g + 1, t - g, xo_next, ag_in, False)

            def exchange(ag_in_ap, xex_ap):
                nc.gpsimd.collective_compute(
                    "AllGather", OP.bypass, replica_groups=rg,
                    ins=[ag_in_ap[:]], outs=[xex_ap[:]])

            # ---------------- program ----------------
            import os
            KREP = int(os.environ.get("KREPEAT", "1"))
            for _rep in range(KREP):
                if _rep > 0:
                    nc.sync.dma_start(
                        out=xoA[:],
                        in_=x_own[:].rearrange("(t p) f -> p t f", p=128))
                nc.sync.dma_start(out=xex0[:], in_=x_pad[:])
                rgcn_layer(0, 0, xex0, xo[0], xo[1], ag0_in, False)
                exchange(ag0_in, xex1)
                rgcn_layer(1, 1, xex1, xo[1], xo[2], ag1_in, False)
                exchange(ag1_in, xex2)
                gat_layer(xo[2], xo[3], ag2_in)
                exchange(ag2_in, xex3)
                rgcn_layer(3, None, xex3, xo[3], None, None, True)

    nc.compile()
    return nc


# ----------------------------------------------------------------------------
# Public API
# ----------------------------------------------------------------------------

_CACHE = {}


def kernel(**inputs):
    cfg = default_cfg()
    N, CORES = cfg["N"], cfg["CORES"]

    key = "k"
    edge_index = np.asarray(inputs["edge_index"])
    edge_type = np.asarray(inputs["edge_type"])
    if key not in _CACHE:
        plan, per_core = build_graph_plan(cfg, edge_index, edge_type)
        nc = build_nc(cfg, plan)
        _CACHE[key] = (nc, plan, per_core)
    nc, plan, per_core = _CACHE[key]

    in_maps = build_in_maps(cfg, inputs, per_core)
    res = run_bass_kernel_spmd(nc, in_maps, list(range(CORES)))
    out = np.concatenate([res.results[c]["out"] for c in range(CORES)], axis=0)
    return out[:N].astype(np.float32)
